# revision 32
# baseline (speedup 1.0000x reference)
"""PTQLinear (smoothquant int8 PTQ linear) on 8 Trainium2 NeuronCores.

Sharding: data-parallel over M rows; weight quantization sharded over N
(N/8 rows per core) and AllGathered as int8; calibration sharded over
rows.  The int8 GEMM runs as bf16 matmuls (ints <= 127 exact in bf16,
fp32 PSUM accumulation exact).

v2 restructure (from baseline-trace analysis, baseline 1.071 ms):
- Cross-core max-reductions use AllGather + local DVE max (~15 us) in
  place of AllReduce (~42 us measured), and every collective bounce
  store / trigger lives on the gpsimd SWDGE queue so it never waits
  behind 2 MB tile loads on the two HWDGE rings.
- Collective order: AG(cal+w colmax) -> AG(x colmax) -> AG(ws) ->
  AG(wq int8).  The baseline queued the x colmax reduction after the
  110 us wq AllGather, stalling x-quant (and the GEMM) until ~450 us.
- x tiles are PE-transposed to [K, M] layout *during* the load phase
  (f32 transposes, PSUM->SBUF copies cast to bf16), so x-quant once
  the input scale is known is just 2 DVE ops per k-slice with
  per-partition scales, done in-place in the transposed buffer -
  no second pass over x from HBM (the baseline reloaded all of x).
- Own-rank GEMM (local bf16 wqt, rank-dynamic output stores via
  partition_id) runs while the wq AllGather is in flight; remote
  ranks (pid+s)&7 stream int8 chunks + bf16 cvt during the GEMM.
"""

from contextlib import ExitStack

import numpy as np

import concourse.bass as bass
import concourse.tile as tile
from concourse import bacc, mybir
from concourse.bass import ts
from concourse.bass_utils import run_bass_kernel_spmd
from concourse.masks import make_identity

F32 = mybir.dt.float32
BF16 = mybir.dt.bfloat16
I8 = mybir.dt.int8
AX = mybir.AxisListType
OP = mybir.AluOpType
ACTF = mybir.ActivationFunctionType

MAGIC = 12582912.0  # 1.5 * 2**23: RNE round-to-int for |v| << 2**22
R127 = float(np.float32(1.0) / np.float32(127.0))


def _fold_partitions_pe(nc, psum, ident, part, res2d, KT):
    """Cross-partition max of a [128, K] bf16 tile (values >= 0) via PE
    transposes of 128x128 blocks + DVE free-dim reduces.  Result layout:
    res2d[p, b] = colmax of channel 128*b + p (f32)."""
    for b in range(KT):
        tps = psum.tile([128, 512], BF16, tag="tpsw", bufs=3)
        nc.tensor.transpose(tps[:, 0:128], part[:, 128 * b : 128 * (b + 1)], ident[:])
        nc.vector.tensor_reduce(res2d[:, b : b + 1], tps[:, 0:128], axis=AX.X, op=OP.max)


def _sqrt_refined(nc, pool, a, out, P, F, iters=2):
    """out = sqrt(a) for [P, F] f32 tiles, ACT seed + Newton via DVE."""
    nc.scalar.activation(out[:], a[:], ACTF.Sqrt)
    for _ in range(iters):
        r = pool.tile([P, F], F32, tag="sqr_r")
        h = pool.tile([P, F], F32, tag="sqr_h")
        nc.vector.reciprocal(r[:], out[:])
        nc.vector.tensor_tensor(h[:], a[:], r[:], op=OP.mult)  # ~ a / y
        nc.vector.tensor_tensor(out[:], out[:], h[:], op=OP.add)
        nc.vector.tensor_scalar(out[:], out[:], 0.5, None, op0=OP.mult)


def _recip_refined(nc, pool, a, out, P, F):
    """out = 1/a (f32), InstReciprocal + one Newton step."""
    r0 = pool.tile([P, F], F32, tag="rcp_r0")
    u = pool.tile([P, F], F32, tag="rcp_u")
    t = pool.tile([P, F], F32, tag="rcp_t")
    nc.vector.reciprocal(r0[:], a[:])
    nc.vector.tensor_tensor(u[:], a[:], r0[:], op=OP.mult)
    nc.vector.tensor_tensor(t[:], r0[:], u[:], op=OP.mult)
    # out = 2*r0 - r0*u
    nc.vector.scalar_tensor_tensor(out[:], r0[:], 2.0, t[:], op0=OP.mult, op1=OP.subtract)


def _div127(nc, pool, num, out, P, F):
    """out = correctly-rounded num / 127 (Newton residual correction)."""
    q0 = pool.tile([P, F], F32, tag="divq0")
    e = pool.tile([P, F], F32, tag="dive")
    nc.vector.tensor_scalar(q0[:], num[:], R127, None, op0=OP.mult)
    nc.vector.scalar_tensor_tensor(e[:], q0[:], -127.0, num[:], op0=OP.mult, op1=OP.add)
    nc.vector.scalar_tensor_tensor(out[:], e[:], R127, q0[:], op0=OP.mult, op1=OP.add)


def build_bass(M, K, N, CAL, n_cores):
    C = n_cores
    MC, NC, CALC = M // C, N // C, CAL // C
    MT, NWT, CT, KT = MC // 128, NC // 128, CALC // 128, K // 128
    KP, NP = K // 128, N // 128
    assert MC % 128 == 0 and NC % 128 == 0 and CALC % 128 == 0 and K % 128 == 0

    nc = bacc.Bacc(None, num_devices=C)
    groups = [list(range(C))]

    x_h = nc.dram_tensor("x", [MC, K], F32, kind="ExternalInput")
    w_h = nc.dram_tensor("w", [NC, K], F32, kind="ExternalInput")
    KC = K // C  # column shard width for cal/wcol colmax
    cal_h = nc.dram_tensor("cal", [CAL, KC], F32, kind="ExternalInput")
    wcol_h = nc.dram_tensor("wcol", [N, KC], F32, kind="ExternalInput")
    bias_h = nc.dram_tensor("bias", [N], F32, kind="ExternalInput")
    out_h = nc.dram_tensor("out", [MC, N], F32, kind="ExternalOutput")

    with tile.TileContext(nc) as tc:
        with ExitStack() as octx:
            dram = octx.enter_context(tc.tile_pool(name="dram", bufs=1, space="DRAM"))
            smalls = octx.enter_context(tc.tile_pool(name="smalls", bufs=1))
            psum = octx.enter_context(tc.tile_pool(name="psum", bufs=1, space="PSUM"))

            # internal DRAM
            NB = KC // 128  # 2D fold width of a column shard
            cc_sm_in = dram.tile([128, NB], F32)
            cc_sm_out = dram.tile([C, 128, NB], F32, addr_space="Shared")
            cc_x_in = dram.tile([128, KP], F32)
            cc_x_out = dram.tile([C, 128, KP], F32, addr_space="Shared")
            wq_mine_d = dram.tile([NWT, K, 128], I8)
            wq_all_d = dram.tile([C, NWT, K, 128], I8, addr_space="Shared")
            ws_mine_d = dram.tile([NC], F32)
            ws_all_d = dram.tile([C, NC], F32, addr_space="Shared")
            smooth_d = dram.tile([K], F32)
            rs_d = dram.tile([1, 1], F32)
            s_bcd = dram.tile([1, 1], F32)
            pv_d = dram.tile([N], F32)

            ident = smalls.tile([128, 128], BF16, tag="ident")
            make_identity(nc, ident[:])
            identf = smalls.tile([128, 128], F32, tag="identf")
            make_identity(nc, identf[:])

            pid_sync = nc.sync.partition_id()
            pid_scalar = nc.scalar.partition_id()

            # persistent SBUF (64 KB/partition): transposed x, quantized
            # in place later (xqt aliases xT)
            xT_ctx = ExitStack()
            p_xT = xT_ctx.enter_context(tc.tile_pool(name="p_xT", bufs=1))
            xT = p_xT.tile([128, KT, MC], BF16, tag="xT")
            wp_ctx = ExitStack()
            wpool2 = wp_ctx.enter_context(tc.tile_pool(name="wpool2", bufs=2))

            # ---------------- Phase L: loads + partials + transposes ------
            # cal and the weight colmax are COLUMN-sharded (each core owns
            # K/C channels over all rows), so smooth for those channels is
            # computed locally and exactly - the cross-core collective
            # shrinks to a 2 KB AllGather of the finished smooth slices.
            KH = K // 2
            HBT = KH // 128  # 128-col blocks per half

            lctx = ExitStack()
            ld_small = lctx.enter_context(tc.tile_pool(name="ld_small", bufs=4))
            ld_half = lctx.enter_context(tc.tile_pool(name="ld_half", bufs=4))
            abspool = lctx.enter_context(tc.tile_pool(name="abspool", bufs=2))
            p_parts = lctx.enter_context(tc.tile_pool(name="p_parts", bufs=1))
            cal_cp = p_parts.tile([128, KC], F32, tag="cal_cp")
            w_cp = p_parts.tile([128, KC], F32, tag="w_cp")
            xmax_p = p_parts.tile([128, K], BF16, tag="xmax_p")
            xmin_p = p_parts.tile([128, K], BF16, tag="xmin_p")
            xbfp = lctx.enter_context(tc.tile_pool(name="xbfp", bufs=3))

            seq = 0

            def col_acc(part, src_h, i, first):
                nonlocal seq
                t_ld = ld_small.tile([128, KC], F32, tag="ld_s")
                eng = nc.sync if seq % 2 == 0 else nc.scalar
                seq += 1
                eng.dma_start(t_ld[:], src_h[128 * i : 128 * (i + 1), :])
                a = abspool.tile([128, KC], F32, tag="abs_tmp")
                nc.scalar.activation(a[:], t_ld[:], ACTF.Abs)
                if first:
                    nc.vector.tensor_copy(part[:], a[:])
                else:
                    nc.vector.tensor_tensor(part[:], part[:], a[:], op=OP.max)

            def fold_small(part, res2d):
                for b in range(NB):
                    tps = psum.tile([128, 512], F32, tag="tps", bufs=2)
                    nc.tensor.transpose(
                        tps[:, 0:128], part[:, 128 * b : 128 * (b + 1)], identf[:])
                    nc.vector.tensor_reduce(
                        res2d[:, b : b + 1], tps[:, 0:128], axis=AX.X, op=OP.max)

            with nc.named_scope("smooth_local"):
                for i in range(CAL // 128):
                    col_acc(cal_cp, cal_h, i, i == 0)
                for i in range(N // 128):
                    col_acc(w_cp, wcol_h, i, i == 0)
                calc2 = smalls.tile([128, NB], F32, tag="calc2")
                wc2 = smalls.tile([128, NB], F32, tag="wc2")
                fold_small(cal_cp, calc2)
                fold_small(w_cp, wc2)
                nc.vector.tensor_scalar(calc2[:], calc2[:], 1e-4, None, op0=OP.max)
                nc.vector.tensor_scalar(wc2[:], wc2[:], 1e-4, None, op0=OP.max)
                sa = smalls.tile([128, NB], F32, tag="sa")
                sw = smalls.tile([128, NB], F32, tag="sw")
                _sqrt_refined(nc, smalls, calc2, sa, 128, NB)
                _sqrt_refined(nc, smalls, wc2, sw, 128, NB)
                rsw = smalls.tile([128, NB], F32, tag="rsw")
                _recip_refined(nc, smalls, sw, rsw, 128, NB)
                sm_loc = smalls.tile([128, NB], F32, tag="sm_loc")
                nc.vector.tensor_tensor(sm_loc[:], sa[:], rsw[:], op=OP.mult)
                nc.vector.tensor_scalar(sm_loc[:], sm_loc[:], 4.0, 0.25,
                                        op0=OP.min, op1=OP.max)
                nc.gpsimd.dma_start(cc_sm_in[:], sm_loc[:])
                nc.gpsimd.collective_compute(
                    "AllGather", OP.bypass, replica_groups=groups,
                    ins=[cc_sm_in[:]], outs=[cc_sm_out[:]],
                )

            # prefetch first two w-quant tiles into their own pool (fresh
            # address space - no WAR wait on the load-ring drains)
            wts = []
            for i in range(2):
                wt = wpool2.tile([128, K], F32, tag="w_t2")
                weng = nc.sync if i % 2 == 0 else nc.scalar
                weng.dma_start(wt[:], w_h[128 * i : 128 * (i + 1), :])
                wts.append(wt)

            # x halves: load f32, ACT cast -> bf16, DVE max+min colmax
            # partials, PE transpose (bf16) -> xT
            with nc.named_scope("x_load_T"):
                for i in range(MT):
                    for h in range(2):
                        t_ld = ld_half.tile([128, KH], F32, tag="ld_h")
                        eng = nc.sync if seq % 2 == 0 else nc.scalar
                        seq += 1
                        eng.dma_start(
                            t_ld[:],
                            x_h[128 * i : 128 * (i + 1), KH * h : KH * (h + 1)])
                        xb = xbfp.tile([128, KH], BF16, tag="xb")
                        nc.scalar.activation(xb[:], t_ld[:], ACTF.Copy)
                        mx = xmax_p[:, KH * h : KH * (h + 1)]
                        mn = xmin_p[:, KH * h : KH * (h + 1)]
                        if i == 0:
                            nc.vector.tensor_copy(mx, xb[:])
                            nc.vector.tensor_copy(mn, xb[:])
                        else:
                            nc.vector.tensor_tensor(mx, mx, xb[:], op=OP.max)
                            nc.vector.tensor_tensor(mn, mn, xb[:], op=OP.min)
                        for g in range(HBT // 4):
                            tps = psum.tile([128, 512], BF16, tag="tpsw", bufs=3)
                            for q in range(4):
                                k = 4 * g + q
                                nc.tensor.transpose(
                                    tps[:, 128 * q : 128 * (q + 1)],
                                    xb[:, 128 * k : 128 * (k + 1)], ident[:])
                            kt0 = HBT * h + 4 * g
                            dst = xT[:, kt0 : kt0 + 4, 128 * i : 128 * (i + 1)]
                            srcv = tps[:].rearrange("p (a b) -> p a b", a=4)
                            if g % 2 == 0:
                                nc.scalar.copy(dst, srcv)
                            else:
                                nc.vector.tensor_copy(dst, srcv)

                # combine |max| / |min| and fold (cc store + AG emitted
                # after the smooth section to keep the gpsimd queue clean)
                nc.vector.scalar_tensor_tensor(
                    xmax_p[:], xmin_p[:], -1.0, xmax_p[:], op0=OP.mult, op1=OP.max)
                xcol2d = smalls.tile([128, KP], F32, tag="xcol2d")
                _fold_partitions_pe(nc, psum, ident, xmax_p, xcol2d, KT)
            lctx.close()
            # remote-chunk/wqt ring opened here (before p_sbc) so pool
            # releases stay stack-ordered; first allocation is in phase C
            wchp_ctx = ExitStack()
            wchp = wchp_ctx.enter_context(tc.tile_pool(name="wchp", bufs=2))

            # ---------------- B1: assemble global smooth -------------------
            with nc.named_scope("smooth"):
                smooth2d = smalls.tile([128, KP], F32, tag="smooth2d")
                nc.gpsimd.dma_start(
                    smooth2d[:].rearrange("p (c b) -> p c b", c=C),
                    cc_sm_out[:].rearrange("c p b -> p c b"),
                )
                it2d = smalls.tile([128, KP], F32, tag="it2d")
                _recip_refined(nc, smalls, smooth2d, it2d, 128, KP)
                nc.gpsimd.dma_start(
                    smooth_d[:].rearrange("(f p) -> p f", p=128), smooth2d[:])
            # smooth_bc broadcast must also beat the x-chain gpsimd ops
            sbc_ctx = ExitStack()
            p_sbc = sbc_ctx.enter_context(tc.tile_pool(name="p_sbc", bufs=1))
            smooth_bc = p_sbc.tile([128, K], F32, tag="smooth_bc")
            nc.gpsimd.dma_start(
                smooth_bc[:],
                smooth_d[:].rearrange("(a k) -> a k", a=1).broadcast_to([128, K]),
            )
            with nc.named_scope("x_colmax_ag"):
                nc.gpsimd.dma_start(cc_x_in[:], xcol2d[:])
                nc.gpsimd.collective_compute(
                    "AllGather", OP.bypass, replica_groups=groups,
                    ins=[cc_x_in[:]], outs=[cc_x_out[:]],
                )

            # ---------------- C: weight quant + transpose + AG ------------
            # wqt shares the wchp ring with the remote GEMM's bf16 chunks:
            # slot 0 holds the local transposed-quantized weights until the
            # own-rank GEMM finishes, then the remote chunks reclaim it.
            wqt = wchp.tile([128, KT, NC], BF16, tag="wch")
            cctx = ExitStack()
            with nc.named_scope("w_quant"):
                p_wq8 = cctx.enter_context(tc.tile_pool(name="p_wq8", bufs=2))
                wqpool = cctx.enter_context(tc.tile_pool(name="wqpool", bufs=1))
                for i in range(NWT):
                    if i < 2:
                        wt = wts[i]
                    else:
                        wt = wpool2.tile([128, K], F32, tag="w_t2")
                        weng = nc.sync if i % 2 == 0 else nc.scalar
                        weng.dma_start(wt[:], w_h[128 * i : 128 * (i + 1), :])
                    nc.vector.tensor_tensor(wt[:], wt[:], smooth_bc[:], op=OP.mult)
                    ws_raw = smalls.tile([128, 1], F32, tag="ws_raw")
                    nc.vector.tensor_reduce(ws_raw[:], wt[:], axis=AX.X, op=OP.max,
                                            apply_absolute_value=True)
                    ws = smalls.tile([128, 1], F32, tag="ws")
                    _div127(nc, smalls, ws_raw, ws, 128, 1)
                    nc.vector.tensor_scalar(ws[:], ws[:], 1e-8, None, op0=OP.max)
                    rws = smalls.tile([128, 1], F32, tag="rws")
                    _recip_refined(nc, smalls, ws, rws, 128, 1)
                    nc.scalar.activation(wt[:], wt[:], ACTF.Copy, scale=rws[:])
                    wq = wqpool.tile([128, K], BF16, tag="wq")
                    nc.vector.tensor_scalar(wq[:], wt[:], MAGIC, MAGIC,
                                            op0=OP.add, op1=OP.subtract)
                    for g in range(KT // 4):
                        tps = psum.tile([128, 512], BF16, tag="tpsw", bufs=3)
                        for q in range(4):
                            k = 4 * g + q
                            nc.tensor.transpose(
                                tps[:, 128 * q : 128 * (q + 1)],
                                wq[:, 128 * k : 128 * (k + 1)], ident[:])
                        dst = wqt[:, 4 * g : 4 * g + 4, 128 * i : 128 * (i + 1)]
                        srcv = tps[:].rearrange("p (a b) -> p a b", a=4)
                        if g % 2 == 0:
                            nc.vector.tensor_copy(dst, srcv)
                        else:
                            nc.scalar.copy(dst, srcv)
                    nc.scalar.dma_start(
                        ws_mine_d[128 * i : 128 * (i + 1)]
                        .rearrange("(p f) -> p f", p=128),
                        ws[:],
                    )
                    # int8 cast + store for this n-tile immediately - the
                    # wq AllGather can trigger right after tile 3 lands
                    q8 = p_wq8.tile([128, KT, 128], I8, tag="wq8")
                    if i % 2 == 0:
                        nc.vector.tensor_copy(q8[:], wqt[:, :, 128 * i : 128 * (i + 1)])
                    else:
                        nc.scalar.copy(q8[:], wqt[:, :, 128 * i : 128 * (i + 1)])
                    seng = nc.sync if i % 2 == 0 else nc.scalar
                    seng.dma_start(
                        wq_mine_d[i].rearrange("(kt p) c -> p kt c", p=128), q8[:]
                    )
            # ---------------- B2: input scale s + per-channel c -----------
            # emitted before the ws/wq AllGather triggers so its gpsimd
            # DMAs are not head-blocked behind them in the queue
            with nc.named_scope("input_scale"):
                gx = smalls.tile([128, C * KP], F32, tag="gx")
                nc.gpsimd.dma_start(
                    gx[:].rearrange("p (c f) -> p c f", c=C),
                    cc_x_out[:].rearrange("c p f -> p c f"),
                )
                xcol_g = smalls.tile([128, KP], F32, tag="xcol_g")
                nc.vector.tensor_copy(xcol_g[:], gx[:, 0:KP])
                for c in range(1, C):
                    nc.vector.tensor_tensor(
                        xcol_g[:], xcol_g[:], gx[:, KP * c : KP * (c + 1)], op=OP.max)
                am_t = smalls.tile([128, KP], F32, tag="am_t")
                nc.vector.tensor_tensor(am_t[:], xcol_g[:], it2d[:], op=OP.mult)
                am_col = smalls.tile([128, 1], F32, tag="am_col")
                nc.vector.tensor_reduce(am_col[:], am_t[:], axis=AX.X, op=OP.max,
                                        apply_absolute_value=True)
                am_row = smalls.tile([1, 128], F32, tag="am_row")
                nc.gpsimd.dma_start(am_row[:], am_col[:])
                amax = smalls.tile([1, 1], F32, tag="amax")
                nc.vector.tensor_reduce(amax[:], am_row[:], axis=AX.X, op=OP.max)

                s_t = smalls.tile([1, 1], F32, tag="s_t")
                _div127(nc, smalls, amax, s_t, 1, 1)
                nc.vector.tensor_scalar(s_t[:], s_t[:], 1e-8, None, op0=OP.max)
                rs_t = smalls.tile([1, 1], F32, tag="rs_t")
                _recip_refined(nc, smalls, s_t, rs_t, 1, 1)
                nc.gpsimd.dma_start(rs_d[:], rs_t[:])
                nc.gpsimd.dma_start(s_bcd[:], s_t[:])
                rs_bc = smalls.tile([128, 1], F32, tag="rs_bc")
                nc.gpsimd.dma_start(rs_bc[:], rs_d[:].broadcast_to([128, 1]))
                # c2d[p, f] = it[128f + p] / s  (matches xT channel layout)
                c2d = smalls.tile([128, KP], F32, tag="c2d")
                nc.vector.tensor_scalar(c2d[:], it2d[:], rs_bc[:], None, op0=OP.mult)

            # ---------------- D: x quant, in place in xT ------------------
            # ACT does the per-partition scale multiply, DVE the exact
            # round-to-int; runs while the wq AllGather is in flight
            with nc.named_scope("x_quant"):
                for kt in range(KT):
                    y = smalls.tile([128, MC], F32, tag="xq_y", bufs=2)
                    nc.scalar.activation(
                        y[:], xT[:, kt, :], ACTF.Copy, scale=c2d[:, kt : kt + 1])
                    nc.vector.tensor_scalar(
                        xT[:, kt, :], y[:], MAGIC, MAGIC, op0=OP.add, op1=OP.subtract)
            xqt = xT  # quantized in place

            nc.gpsimd.collective_compute(
                "AllGather", OP.bypass, replica_groups=groups,
                ins=[ws_mine_d[:]], outs=[ws_all_d[:]],
            )
            nc.gpsimd.collective_compute(
                "AllGather", OP.bypass, replica_groups=groups,
                ins=[wq_mine_d[:]], outs=[wq_all_d[:]],
            )
            cctx.close()
            sbc_ctx.close()

            # pv = s * ws over all N, written to DRAM for rank-sliced reads
            with nc.named_scope("pv"):
                s_bc = smalls.tile([128, 1], F32, tag="s_bc")
                nc.gpsimd.dma_start(s_bc[:], s_bcd[:].broadcast_to([128, 1]))
                ws2d = smalls.tile([128, NP], F32, tag="ws2d")
                nc.scalar.dma_start(
                    ws2d[:], ws_all_d[:].rearrange("c (pc f) -> (c pc) f", f=NP)
                )
                pv2d = smalls.tile([128, NP], F32, tag="pv2d")
                nc.vector.tensor_scalar(pv2d[:], ws2d[:], s_bc[:], None, op0=OP.mult)
                nc.gpsimd.dma_start(pv_d[:].rearrange("(p f) -> p f", p=128), pv2d[:])

            # ---------------- E: GEMM -------------------------------------
            def gemm_rank(rank_sync, rank_scalar, rhs_t, pvb_pool, ostage, tag):
                """GEMM over one rank's NC output columns; rank_* are
                per-engine ScalarValue column-block indices."""
                own_pv = pvb_pool.tile([128, NC], F32, tag="pvb", bufs=4)
                own_bias = pvb_pool.tile([128, NC], F32, tag="pvb", bufs=4)
                nc.scalar.dma_start(
                    own_pv[:],
                    pv_d[ts(rank_scalar, NC)].rearrange("(a n) -> a n", a=1)
                    .broadcast_to([128, NC]),
                )
                nc.scalar.dma_start(
                    own_bias[:],
                    bias_h[ts(rank_scalar, NC)].rearrange("(a n) -> a n", a=1)
                    .broadcast_to([128, NC]),
                )
                for m in range(MT):
                    ps = psum.tile([128, NC], F32, tag="ps", bufs=3)
                    for k in range(KT):
                        nc.tensor.matmul(
                            ps[:],
                            lhsT=xqt[:, k, 128 * m : 128 * (m + 1)],
                            rhs=rhs_t[:, k, :],
                            start=(k == 0),
                            stop=(k == KT - 1),
                        )
                    o = ostage.tile([128, NC], F32, tag="o")
                    nc.vector.tensor_tensor(o[:], ps[:], own_pv[:], op=OP.mult)
                    nc.vector.tensor_tensor(o[:], o[:], own_bias[:], op=OP.add)
                    nc.sync.dma_start(
                        out_h[128 * m : 128 * (m + 1), ts(rank_sync, NC)], o[:]
                    )

            with tc.tile_pool(name="ostage", bufs=3) as ostage, \
                 tc.tile_pool(name="pvb", bufs=1) as pvb_pool:
                # own rank first: local bf16 wqt, overlaps the wq
                # AllGather; k-outer in m-groups of 3 so the matmuls track
                # the x-quant rounds instead of waiting for all of xqt
                with nc.named_scope("own_gemm"):
                    own_pv = pvb_pool.tile([128, NC], F32, tag="pvb", bufs=4)
                    own_bias = pvb_pool.tile([128, NC], F32, tag="pvb", bufs=4)
                    nc.scalar.dma_start(
                        own_pv[:],
                        pv_d[ts(pid_scalar, NC)].rearrange("(a n) -> a n", a=1)
                        .broadcast_to([128, NC]),
                    )
                    nc.scalar.dma_start(
                        own_bias[:],
                        bias_h[ts(pid_scalar, NC)].rearrange("(a n) -> a n", a=1)
                        .broadcast_to([128, NC]),
                    )
                    for m0 in range(0, MT, 3):
                        ms = list(range(m0, min(m0 + 3, MT)))
                        pss = {}
                        for m in ms:
                            ps_m = psum.tile([128, NC], F32, tag="ps", bufs=3)
                            pss[m] = ps_m
                        for k in range(KT):
                            for m in ms:
                                nc.tensor.matmul(
                                    pss[m][:],
                                    lhsT=xqt[:, k, 128 * m : 128 * (m + 1)],
                                    rhs=wqt[:, k, :],
                                    start=(k == 0),
                                    stop=(k == KT - 1),
                                )
                        for m in ms:
                            o = ostage.tile([128, NC], F32, tag="o")
                            nc.vector.tensor_tensor(o[:], pss[m][:], own_pv[:], op=OP.mult)
                            nc.vector.tensor_tensor(o[:], o[:], own_bias[:], op=OP.add)
                            nc.sync.dma_start(
                                out_h[128 * m : 128 * (m + 1), ts(pid_sync, NC)], o[:]
                            )
                # remote ranks stream from the gathered int8 buffer

                with nc.named_scope("remote_gemm"), \
                     tc.tile_pool(name="ch8p", bufs=1) as ch8p:
                    for s in range(1, C):
                        rank_sync = (pid_sync + s) & (C - 1)
                        rank_scalar = (pid_scalar + s) & (C - 1)
                        ch8 = ch8p.tile([128, KT, NWT, 128], I8, tag="wch8")
                        for k in range(KT):
                            ceng = nc.scalar if k % 2 == 0 else nc.sync
                            rk = rank_scalar if k % 2 == 0 else rank_sync
                            ceng.dma_start(
                                ch8[:, k, :, :],
                                wq_all_d[bass.ds(rk, 1), :, 128 * k : 128 * (k + 1), :]
                                .rearrange("a i p c -> p (a i) c"),
                            )
                        ch = wchp.tile([128, KT, NC], BF16, tag="wch")
                        for k2 in range(KT // 2):
                            src = ch8[:, 2 * k2 : 2 * k2 + 2, :, :].rearrange(
                                "p a i c -> p a (i c)")
                            dst = ch[:, 2 * k2 : 2 * k2 + 2, :]
                            if k2 % 2 == 0:
                                nc.vector.tensor_copy(dst, src)
                            else:
                                nc.scalar.copy(dst, src)
                        gemm_rank(rank_sync, rank_scalar, ch, pvb_pool, ostage, "rem")
            wchp_ctx.close()
            wp_ctx.close()
            xT_ctx.close()

    nc.finalize()
    return nc


class _Built:
    cache = {}


def _get_built(M, K, N, CAL, n_cores):
    key = (M, K, N, CAL, n_cores)
    if key not in _Built.cache:
        _Built.cache[key] = build_bass(M, K, N, CAL, n_cores)
    return _Built.cache[key]


def make_in_maps(x, weight, bias, calibration, n_cores):
    C = n_cores
    M = x.shape[0]
    K = x.shape[1]
    N = weight.shape[0]
    CAL = calibration.shape[0]
    MC, NC, CALC = M // C, N // C, CAL // C
    x = np.ascontiguousarray(x, dtype=np.float32)
    weight = np.ascontiguousarray(weight, dtype=np.float32)
    bias = np.ascontiguousarray(bias, dtype=np.float32)
    calibration = np.ascontiguousarray(calibration, dtype=np.float32)
    KC = K // C
    return [
        {
            "x": x[c * MC : (c + 1) * MC],
            "w": weight[c * NC : (c + 1) * NC],
            "wcol": np.ascontiguousarray(weight[:, c * KC : (c + 1) * KC]),
            "cal": np.ascontiguousarray(calibration[:, c * KC : (c + 1) * KC]),
            "bias": bias,
        }
        for c in range(C)
    ]


def kernel(x, weight, bias, calibration):
    n_cores = 8
    M, K = x.shape
    N = weight.shape[0]
    CAL = calibration.shape[0]
    nc = _get_built(M, K, N, CAL, n_cores)
    in_maps = make_in_maps(x, weight, bias, calibration, n_cores)
    res = run_bass_kernel_spmd(nc, in_maps, list(range(n_cores)))
    out = np.concatenate([res.results[c]["out"] for c in range(n_cores)], axis=0)
    return out.astype(np.float32)


# revision 33
# speedup vs baseline: 1.0760x; 1.0760x over previous
"""PTQLinear (smoothquant int8 PTQ linear) on 8 Trainium2 NeuronCores.

Sharding: data-parallel over M rows; weight quantization sharded over N
(N/8 rows per core) and AllGathered as int8; calibration sharded over
rows.  The int8 GEMM runs as bf16 matmuls (ints <= 127 exact in bf16,
fp32 PSUM accumulation exact).

v2 restructure (from baseline-trace analysis, baseline 1.071 ms):
- Cross-core max-reductions use AllGather + local DVE max (~15 us) in
  place of AllReduce (~42 us measured), and every collective bounce
  store / trigger lives on the gpsimd SWDGE queue so it never waits
  behind 2 MB tile loads on the two HWDGE rings.
- Collective order: AG(cal+w colmax) -> AG(x colmax) -> AG(ws) ->
  AG(wq int8).  The baseline queued the x colmax reduction after the
  110 us wq AllGather, stalling x-quant (and the GEMM) until ~450 us.
- x tiles are PE-transposed to [K, M] layout *during* the load phase
  (f32 transposes, PSUM->SBUF copies cast to bf16), so x-quant once
  the input scale is known is just 2 DVE ops per k-slice with
  per-partition scales, done in-place in the transposed buffer -
  no second pass over x from HBM (the baseline reloaded all of x).
- Own-rank GEMM (local bf16 wqt, rank-dynamic output stores via
  partition_id) runs while the wq AllGather is in flight; remote
  ranks (pid+s)&7 stream int8 chunks + bf16 cvt during the GEMM.
"""

from contextlib import ExitStack

import numpy as np

import concourse.bass as bass
import concourse.tile as tile
from concourse import bacc, mybir
from concourse.bass import ts
from concourse.bass_utils import run_bass_kernel_spmd
from concourse.masks import make_identity

F32 = mybir.dt.float32
BF16 = mybir.dt.bfloat16
I8 = mybir.dt.int8
AX = mybir.AxisListType
OP = mybir.AluOpType
ACTF = mybir.ActivationFunctionType

MAGIC = 12582912.0  # 1.5 * 2**23: RNE round-to-int for |v| << 2**22
R127 = float(np.float32(1.0) / np.float32(127.0))


def _fold_partitions_pe(nc, psum, ident, part, res2d, KT):
    """Cross-partition max of a [128, K] bf16 tile (values >= 0) via PE
    transposes of 128x128 blocks + DVE free-dim reduces.  Result layout:
    res2d[p, b] = colmax of channel 128*b + p (f32)."""
    for b in range(KT):
        tps = psum.tile([128, 512], BF16, tag="tpsw", bufs=2)
        nc.tensor.transpose(tps[:, 0:128], part[:, 128 * b : 128 * (b + 1)], ident[:])
        nc.vector.tensor_reduce(res2d[:, b : b + 1], tps[:, 0:128], axis=AX.X, op=OP.max)


def _sqrt_refined(nc, pool, a, out, P, F, iters=2):
    """out = sqrt(a) for [P, F] f32 tiles, ACT seed + Newton via DVE."""
    nc.scalar.activation(out[:], a[:], ACTF.Sqrt)
    for _ in range(iters):
        r = pool.tile([P, F], F32, tag="sqr_r")
        h = pool.tile([P, F], F32, tag="sqr_h")
        nc.vector.reciprocal(r[:], out[:])
        nc.vector.tensor_tensor(h[:], a[:], r[:], op=OP.mult)  # ~ a / y
        nc.vector.tensor_tensor(out[:], out[:], h[:], op=OP.add)
        nc.vector.tensor_scalar(out[:], out[:], 0.5, None, op0=OP.mult)


def _recip_refined(nc, pool, a, out, P, F):
    """out = 1/a (f32), InstReciprocal + one Newton step."""
    r0 = pool.tile([P, F], F32, tag="rcp_r0")
    u = pool.tile([P, F], F32, tag="rcp_u")
    t = pool.tile([P, F], F32, tag="rcp_t")
    nc.vector.reciprocal(r0[:], a[:])
    nc.vector.tensor_tensor(u[:], a[:], r0[:], op=OP.mult)
    nc.vector.tensor_tensor(t[:], r0[:], u[:], op=OP.mult)
    # out = 2*r0 - r0*u
    nc.vector.scalar_tensor_tensor(out[:], r0[:], 2.0, t[:], op0=OP.mult, op1=OP.subtract)


def _div127(nc, pool, num, out, P, F):
    """out = correctly-rounded num / 127 (Newton residual correction)."""
    q0 = pool.tile([P, F], F32, tag="divq0")
    e = pool.tile([P, F], F32, tag="dive")
    nc.vector.tensor_scalar(q0[:], num[:], R127, None, op0=OP.mult)
    nc.vector.scalar_tensor_tensor(e[:], q0[:], -127.0, num[:], op0=OP.mult, op1=OP.add)
    nc.vector.scalar_tensor_tensor(out[:], e[:], R127, q0[:], op0=OP.mult, op1=OP.add)


def build_bass(M, K, N, CAL, n_cores):
    C = n_cores
    MC, NC, CALC = M // C, N // C, CAL // C
    MT, NWT, CT, KT = MC // 128, NC // 128, CALC // 128, K // 128
    KP, NP = K // 128, N // 128
    assert MC % 128 == 0 and NC % 128 == 0 and CALC % 128 == 0 and K % 128 == 0

    nc = bacc.Bacc(None, num_devices=C)
    groups = [list(range(C))]

    x_h = nc.dram_tensor("x", [MC, K], F32, kind="ExternalInput")
    w_h = nc.dram_tensor("w", [NC, K], F32, kind="ExternalInput")
    cal_h = nc.dram_tensor("cal", [CALC, K], F32, kind="ExternalInput")
    bias_h = nc.dram_tensor("bias", [N], F32, kind="ExternalInput")
    out_h = nc.dram_tensor("out", [MC, N], F32, kind="ExternalOutput")

    with tile.TileContext(nc) as tc:
        with ExitStack() as octx:
            dram = octx.enter_context(tc.tile_pool(name="dram", bufs=1, space="DRAM"))
            smalls = octx.enter_context(tc.tile_pool(name="smalls", bufs=1))
            psum = octx.enter_context(tc.tile_pool(name="psum", bufs=1, space="PSUM"))

            # internal DRAM
            cc_cw_in = dram.tile([2, 128, KP], F32)
            cc_cw_out = dram.tile([C, 2, 128, KP], F32, addr_space="Shared")
            cc_x_in = dram.tile([128, KP], F32)
            cc_x_out = dram.tile([C, 128, KP], F32, addr_space="Shared")
            wq_mine_d = dram.tile([NWT, K, 128], I8)
            wq_all_d = dram.tile([C, NWT, K, 128], I8, addr_space="Shared")
            ws_mine_d = dram.tile([NC], F32)
            ws_all_d = dram.tile([C, NC], F32, addr_space="Shared")
            smooth_d = dram.tile([K], F32)
            rs_d = dram.tile([1, 1], F32)
            s_bcd = dram.tile([1, 1], F32)
            pv_d = dram.tile([N], F32)

            ident = smalls.tile([128, 128], BF16, tag="ident")
            make_identity(nc, ident[:])
            identf = smalls.tile([128, 128], F32, tag="identf")
            make_identity(nc, identf[:])

            pid_sync = nc.sync.partition_id()
            pid_scalar = nc.scalar.partition_id()

            # persistent SBUF (64 KB/partition): transposed x, quantized
            # in place later (xqt aliases xT)
            xT_ctx = ExitStack()
            p_xT = xT_ctx.enter_context(tc.tile_pool(name="p_xT", bufs=1))
            xT = p_xT.tile([128, KT, MC], BF16, tag="xT")
            wp_ctx = ExitStack()
            wpool2 = wp_ctx.enter_context(tc.tile_pool(name="wpool2", bufs=2))

            # ---------------- Phase L: loads + partials + transposes ------
            # half-tiles ([128, K/2], 1 MB) through a 4-slot ring keep the
            # two HWDGE queues streaming at full rate; 2 MB tiles with a
            # 2-slot ring measured ~18 us/tile (slot-lifetime-bound).
            KH = K // 2
            HBT = KH // 128  # 128-col blocks per half

            def acc_half(ldpool, abspool, part, src_h, i, h, seq, first):
                t_ld = ldpool.tile([128, KH], F32, tag="ld_t")
                eng = nc.sync if seq % 2 == 0 else nc.scalar
                eng.dma_start(
                    t_ld[:], src_h[128 * i : 128 * (i + 1), KH * h : KH * (h + 1)])
                a = abspool.tile([128, KH], BF16, tag="abs_tmp")
                nc.scalar.activation(a[:], t_ld[:], ACTF.Abs)
                dstp = part[:, KH * h : KH * (h + 1)]
                if first:
                    nc.vector.tensor_copy(dstp, a[:])
                else:
                    nc.vector.tensor_tensor(dstp, dstp, a[:], op=OP.max)
                return t_ld

            lctx = ExitStack()
            ldpool = lctx.enter_context(tc.tile_pool(name="ldpool", bufs=4))
            abspool = lctx.enter_context(tc.tile_pool(name="abspool", bufs=1))
            p_parts = lctx.enter_context(tc.tile_pool(name="p_parts", bufs=1))
            cal_part = p_parts.tile([128, K], BF16, tag="cal_part")
            w_part = p_parts.tile([128, K], BF16, tag="w_part")
            xcol_part = p_parts.tile([128, K], BF16, tag="xcol_part")

            seq = 0
            with nc.named_scope("cw_colmax"):
                for i in range(CT):
                    for h in range(2):
                        acc_half(ldpool, abspool, cal_part, cal_h, i, h, seq, i == 0)
                        seq += 1
                for i in range(NWT):
                    for h in range(2):
                        acc_half(ldpool, abspool, w_part, w_h, i, h, seq, i == 0)
                        seq += 1
                cal2d = smalls.tile([128, KP], F32, tag="cal2d")
                w2d = smalls.tile([128, KP], F32, tag="w2d")
                _fold_partitions_pe(nc, psum, ident, cal_part, cal2d, KT)
                _fold_partitions_pe(nc, psum, ident, w_part, w2d, KT)
                nc.gpsimd.dma_start(cc_cw_in[0], cal2d[:])
                nc.gpsimd.dma_start(cc_cw_in[1], w2d[:])
                nc.gpsimd.collective_compute(
                    "AllGather", OP.bypass, replica_groups=groups,
                    ins=[cc_cw_in[:]], outs=[cc_cw_out[:]],
                )

            # prefetch first two w-quant tiles into their own pool (fresh
            # address space - no WAR wait on the load-ring drains)
            wts = []
            for i in range(2):
                wt = wpool2.tile([128, K], F32, tag="w_t2")
                weng = nc.sync if i % 2 == 0 else nc.scalar
                weng.dma_start(wt[:], w_h[128 * i : 128 * (i + 1), :])
                wts.append(wt)

            # x tiles: load, abs->colmax partial, PE transpose -> xT (bf16)
            with nc.named_scope("x_load_T"):
                for i in range(MT):
                    for h in range(2):
                        t_ld = acc_half(
                            ldpool, abspool, xcol_part, x_h, i, h, seq, i == 0)
                        seq += 1
                        for g in range(HBT // 4):
                            tps = psum.tile([128, 512], F32, tag="tps", bufs=3)
                            for q in range(4):
                                k = 4 * g + q
                                nc.tensor.transpose(
                                    tps[:, 128 * q : 128 * (q + 1)],
                                    t_ld[:, 128 * k : 128 * (k + 1)], identf[:])
                            kt0 = HBT * h + 4 * g
                            dst = xT[:, kt0 : kt0 + 4, 128 * i : 128 * (i + 1)]
                            srcv = tps[:].rearrange("p (a b) -> p a b", a=4)
                            if g % 4 == 1:
                                nc.scalar.copy(dst, srcv)
                            else:
                                nc.vector.tensor_copy(dst, srcv)

                # fold x colmax (cc store + AG are emitted after the
                # smooth section: the gpsimd queue is FIFO and the store
                # waits on this fold - emitting it earlier head-blocks the
                # gcw/smooth_bc DMAs behind the x chain)
                xcol2d = smalls.tile([128, KP], F32, tag="xcol2d")
                _fold_partitions_pe(nc, psum, ident, xcol_part, xcol2d, KT)
            lctx.close()
            # remote-chunk/wqt ring opened here (before p_sbc) so pool
            # releases stay stack-ordered; first allocation is in phase C
            wchp_ctx = ExitStack()
            wchp = wchp_ctx.enter_context(tc.tile_pool(name="wchp", bufs=2))

            # ---------------- B1: smooth / input_transform ----------------
            with nc.named_scope("smooth"):
                gcw = smalls.tile([128, C * 2 * KP], F32, tag="gcw")
                nc.gpsimd.dma_start(
                    gcw[:].rearrange("p (c t f) -> p c t f", c=C, t=2),
                    cc_cw_out[:].rearrange("c t p f -> p c t f"),
                )
                act_t = smalls.tile([128, KP], F32, tag="act_t")
                wcs_t = smalls.tile([128, KP], F32, tag="wcs_t")
                nc.vector.tensor_copy(act_t[:], gcw[:, 0:KP])
                nc.vector.tensor_copy(wcs_t[:], gcw[:, KP : 2 * KP])
                for c in range(1, C):
                    o = 2 * KP * c
                    nc.vector.tensor_tensor(act_t[:], act_t[:], gcw[:, o : o + KP], op=OP.max)
                    nc.vector.tensor_tensor(wcs_t[:], wcs_t[:], gcw[:, o + KP : o + 2 * KP], op=OP.max)
                nc.vector.tensor_scalar(act_t[:], act_t[:], 1e-4, None, op0=OP.max)
                nc.vector.tensor_scalar(wcs_t[:], wcs_t[:], 1e-4, None, op0=OP.max)

                sa = smalls.tile([128, KP], F32, tag="sa")
                sw = smalls.tile([128, KP], F32, tag="sw")
                _sqrt_refined(nc, smalls, act_t, sa, 128, KP)
                _sqrt_refined(nc, smalls, wcs_t, sw, 128, KP)
                rsw = smalls.tile([128, KP], F32, tag="rsw")
                _recip_refined(nc, smalls, sw, rsw, 128, KP)
                smooth = smalls.tile([128, KP], F32, tag="smooth")
                nc.vector.tensor_tensor(smooth[:], sa[:], rsw[:], op=OP.mult)
                nc.vector.tensor_scalar(smooth[:], smooth[:], 4.0, 0.25, op0=OP.min, op1=OP.max)
                it2d = smalls.tile([128, KP], F32, tag="it2d")
                _recip_refined(nc, smalls, smooth, it2d, 128, KP)
                nc.gpsimd.dma_start(smooth_d[:].rearrange("(f p) -> p f", p=128), smooth[:])
            # smooth_bc broadcast must also beat the x-chain gpsimd ops
            sbc_ctx = ExitStack()
            p_sbc = sbc_ctx.enter_context(tc.tile_pool(name="p_sbc", bufs=1))
            smooth_bc = p_sbc.tile([128, K], F32, tag="smooth_bc")
            nc.gpsimd.dma_start(
                smooth_bc[:],
                smooth_d[:].rearrange("(a k) -> a k", a=1).broadcast_to([128, K]),
            )
            with nc.named_scope("x_colmax_ag"):
                nc.gpsimd.dma_start(cc_x_in[:], xcol2d[:])
                nc.gpsimd.collective_compute(
                    "AllGather", OP.bypass, replica_groups=groups,
                    ins=[cc_x_in[:]], outs=[cc_x_out[:]],
                )

            # ---------------- C: weight quant + transpose + AG ------------
            # wqt shares the wchp ring with the remote GEMM's bf16 chunks:
            # slot 0 holds the local transposed-quantized weights until the
            # own-rank GEMM finishes, then the remote chunks reclaim it.
            wqt = wchp.tile([128, KT, NC], BF16, tag="wch")
            cctx = ExitStack()
            with nc.named_scope("w_quant"):
                p_wq8 = cctx.enter_context(tc.tile_pool(name="p_wq8", bufs=2))
                wqpool = cctx.enter_context(tc.tile_pool(name="wqpool", bufs=1))
                for i in range(NWT):
                    if i < 2:
                        wt = wts[i]
                    else:
                        wt = wpool2.tile([128, K], F32, tag="w_t2")
                        weng = nc.sync if i % 2 == 0 else nc.scalar
                        weng.dma_start(wt[:], w_h[128 * i : 128 * (i + 1), :])
                    nc.vector.tensor_tensor(wt[:], wt[:], smooth_bc[:], op=OP.mult)
                    ws_raw = smalls.tile([128, 1], F32, tag="ws_raw")
                    nc.vector.tensor_reduce(ws_raw[:], wt[:], axis=AX.X, op=OP.max,
                                            apply_absolute_value=True)
                    ws = smalls.tile([128, 1], F32, tag="ws")
                    _div127(nc, smalls, ws_raw, ws, 128, 1)
                    nc.vector.tensor_scalar(ws[:], ws[:], 1e-8, None, op0=OP.max)
                    rws = smalls.tile([128, 1], F32, tag="rws")
                    _recip_refined(nc, smalls, ws, rws, 128, 1)
                    nc.scalar.activation(wt[:], wt[:], ACTF.Copy, scale=rws[:])
                    wq = wqpool.tile([128, K], BF16, tag="wq")
                    nc.vector.tensor_scalar(wq[:], wt[:], MAGIC, MAGIC,
                                            op0=OP.add, op1=OP.subtract)
                    for g in range(KT // 4):
                        tps = psum.tile([128, 512], BF16, tag="tpsw", bufs=2)
                        for q in range(4):
                            k = 4 * g + q
                            nc.tensor.transpose(
                                tps[:, 128 * q : 128 * (q + 1)],
                                wq[:, 128 * k : 128 * (k + 1)], ident[:])
                        dst = wqt[:, 4 * g : 4 * g + 4, 128 * i : 128 * (i + 1)]
                        srcv = tps[:].rearrange("p (a b) -> p a b", a=4)
                        if g % 2 == 0:
                            nc.vector.tensor_copy(dst, srcv)
                        else:
                            nc.scalar.copy(dst, srcv)
                    nc.scalar.dma_start(
                        ws_mine_d[128 * i : 128 * (i + 1)]
                        .rearrange("(p f) -> p f", p=128),
                        ws[:],
                    )
                    # int8 cast + store for this n-tile immediately - the
                    # wq AllGather can trigger right after tile 3 lands
                    q8 = p_wq8.tile([128, KT, 128], I8, tag="wq8")
                    if i % 2 == 0:
                        nc.vector.tensor_copy(q8[:], wqt[:, :, 128 * i : 128 * (i + 1)])
                    else:
                        nc.scalar.copy(q8[:], wqt[:, :, 128 * i : 128 * (i + 1)])
                    seng = nc.sync if i % 2 == 0 else nc.scalar
                    seng.dma_start(
                        wq_mine_d[i].rearrange("(kt p) c -> p kt c", p=128), q8[:]
                    )
            # ---------------- B2: input scale s + per-channel c -----------
            # emitted before the ws/wq AllGather triggers so its gpsimd
            # DMAs are not head-blocked behind them in the queue
            with nc.named_scope("input_scale"):
                gx = smalls.tile([128, C * KP], F32, tag="gx")
                nc.gpsimd.dma_start(
                    gx[:].rearrange("p (c f) -> p c f", c=C),
                    cc_x_out[:].rearrange("c p f -> p c f"),
                )
                xcol_g = smalls.tile([128, KP], F32, tag="xcol_g")
                nc.vector.tensor_copy(xcol_g[:], gx[:, 0:KP])
                for c in range(1, C):
                    nc.vector.tensor_tensor(
                        xcol_g[:], xcol_g[:], gx[:, KP * c : KP * (c + 1)], op=OP.max)
                am_t = smalls.tile([128, KP], F32, tag="am_t")
                nc.vector.tensor_tensor(am_t[:], xcol_g[:], it2d[:], op=OP.mult)
                am_col = smalls.tile([128, 1], F32, tag="am_col")
                nc.vector.tensor_reduce(am_col[:], am_t[:], axis=AX.X, op=OP.max,
                                        apply_absolute_value=True)
                am_row = smalls.tile([1, 128], F32, tag="am_row")
                nc.gpsimd.dma_start(am_row[:], am_col[:])
                amax = smalls.tile([1, 1], F32, tag="amax")
                nc.vector.tensor_reduce(amax[:], am_row[:], axis=AX.X, op=OP.max)

                s_t = smalls.tile([1, 1], F32, tag="s_t")
                _div127(nc, smalls, amax, s_t, 1, 1)
                nc.vector.tensor_scalar(s_t[:], s_t[:], 1e-8, None, op0=OP.max)
                rs_t = smalls.tile([1, 1], F32, tag="rs_t")
                _recip_refined(nc, smalls, s_t, rs_t, 1, 1)
                nc.gpsimd.dma_start(rs_d[:], rs_t[:])
                nc.gpsimd.dma_start(s_bcd[:], s_t[:])
                rs_bc = smalls.tile([128, 1], F32, tag="rs_bc")
                nc.gpsimd.dma_start(rs_bc[:], rs_d[:].broadcast_to([128, 1]))
                # c2d[p, f] = it[128f + p] / s  (matches xT channel layout)
                c2d = smalls.tile([128, KP], F32, tag="c2d")
                nc.vector.tensor_scalar(c2d[:], it2d[:], rs_bc[:], None, op0=OP.mult)

            # ---------------- D: x quant, in place in xT ------------------
            # ACT does the per-partition scale multiply, DVE the exact
            # round-to-int; runs while the wq AllGather is in flight
            with nc.named_scope("x_quant"):
                for kt in range(KT):
                    y = smalls.tile([128, MC], F32, tag="xq_y", bufs=2)
                    nc.scalar.activation(
                        y[:], xT[:, kt, :], ACTF.Copy, scale=c2d[:, kt : kt + 1])
                    nc.vector.tensor_scalar(
                        xT[:, kt, :], y[:], MAGIC, MAGIC, op0=OP.add, op1=OP.subtract)
            xqt = xT  # quantized in place

            nc.gpsimd.collective_compute(
                "AllGather", OP.bypass, replica_groups=groups,
                ins=[ws_mine_d[:]], outs=[ws_all_d[:]],
            )
            nc.gpsimd.collective_compute(
                "AllGather", OP.bypass, replica_groups=groups,
                ins=[wq_mine_d[:]], outs=[wq_all_d[:]],
            )
            cctx.close()
            sbc_ctx.close()

            # pv = s * ws over all N, written to DRAM for rank-sliced reads
            with nc.named_scope("pv"):
                s_bc = smalls.tile([128, 1], F32, tag="s_bc")
                nc.gpsimd.dma_start(s_bc[:], s_bcd[:].broadcast_to([128, 1]))
                ws2d = smalls.tile([128, NP], F32, tag="ws2d")
                nc.scalar.dma_start(
                    ws2d[:], ws_all_d[:].rearrange("c (pc f) -> (c pc) f", f=NP)
                )
                pv2d = smalls.tile([128, NP], F32, tag="pv2d")
                nc.vector.tensor_scalar(pv2d[:], ws2d[:], s_bc[:], None, op0=OP.mult)
                nc.gpsimd.dma_start(pv_d[:].rearrange("(p f) -> p f", p=128), pv2d[:])

            # ---------------- E: GEMM -------------------------------------
            def gemm_rank(rank_sync, rank_scalar, rhs_t, pvb_pool, ostage, tag):
                """GEMM over one rank's NC output columns; rank_* are
                per-engine ScalarValue column-block indices."""
                own_pv = pvb_pool.tile([128, NC], F32, tag="pvb", bufs=4)
                own_bias = pvb_pool.tile([128, NC], F32, tag="pvb", bufs=4)
                nc.scalar.dma_start(
                    own_pv[:],
                    pv_d[ts(rank_scalar, NC)].rearrange("(a n) -> a n", a=1)
                    .broadcast_to([128, NC]),
                )
                nc.scalar.dma_start(
                    own_bias[:],
                    bias_h[ts(rank_scalar, NC)].rearrange("(a n) -> a n", a=1)
                    .broadcast_to([128, NC]),
                )
                for m in range(MT):
                    ps = psum.tile([128, NC], F32, tag="ps", bufs=3)
                    for k in range(KT):
                        nc.tensor.matmul(
                            ps[:],
                            lhsT=xqt[:, k, 128 * m : 128 * (m + 1)],
                            rhs=rhs_t[:, k, :],
                            start=(k == 0),
                            stop=(k == KT - 1),
                        )
                    o = ostage.tile([128, NC], F32, tag="o")
                    nc.vector.tensor_tensor(o[:], ps[:], own_pv[:], op=OP.mult)
                    nc.vector.tensor_tensor(o[:], o[:], own_bias[:], op=OP.add)
                    nc.sync.dma_start(
                        out_h[128 * m : 128 * (m + 1), ts(rank_sync, NC)], o[:]
                    )

            with tc.tile_pool(name="ostage", bufs=3) as ostage, \
                 tc.tile_pool(name="pvb", bufs=1) as pvb_pool:
                # own rank first: local bf16 wqt, overlaps the wq
                # AllGather; k-outer in m-groups of 3 so the matmuls track
                # the x-quant rounds instead of waiting for all of xqt
                with nc.named_scope("own_gemm"):
                    own_pv = pvb_pool.tile([128, NC], F32, tag="pvb", bufs=4)
                    own_bias = pvb_pool.tile([128, NC], F32, tag="pvb", bufs=4)
                    nc.scalar.dma_start(
                        own_pv[:],
                        pv_d[ts(pid_scalar, NC)].rearrange("(a n) -> a n", a=1)
                        .broadcast_to([128, NC]),
                    )
                    nc.scalar.dma_start(
                        own_bias[:],
                        bias_h[ts(pid_scalar, NC)].rearrange("(a n) -> a n", a=1)
                        .broadcast_to([128, NC]),
                    )
                    for m0 in range(0, MT, 3):
                        ms = list(range(m0, min(m0 + 3, MT)))
                        pss = {}
                        for m in ms:
                            ps_m = psum.tile([128, NC], F32, tag="ps", bufs=3)
                            pss[m] = ps_m
                        for k in range(KT):
                            for m in ms:
                                nc.tensor.matmul(
                                    pss[m][:],
                                    lhsT=xqt[:, k, 128 * m : 128 * (m + 1)],
                                    rhs=wqt[:, k, :],
                                    start=(k == 0),
                                    stop=(k == KT - 1),
                                )
                        for m in ms:
                            o = ostage.tile([128, NC], F32, tag="o")
                            nc.vector.tensor_tensor(o[:], pss[m][:], own_pv[:], op=OP.mult)
                            nc.vector.tensor_tensor(o[:], o[:], own_bias[:], op=OP.add)
                            nc.sync.dma_start(
                                out_h[128 * m : 128 * (m + 1), ts(pid_sync, NC)], o[:]
                            )
                # remote ranks stream from the gathered int8 buffer

                with nc.named_scope("remote_gemm"), \
                     tc.tile_pool(name="ch8p", bufs=1) as ch8p:
                    for s in range(1, C):
                        rank_sync = (pid_sync + s) & (C - 1)
                        rank_scalar = (pid_scalar + s) & (C - 1)
                        ch8 = ch8p.tile([128, KT, NWT, 128], I8, tag="wch8")
                        for k in range(KT):
                            ceng = nc.scalar if k % 2 == 0 else nc.sync
                            rk = rank_scalar if k % 2 == 0 else rank_sync
                            ceng.dma_start(
                                ch8[:, k, :, :],
                                wq_all_d[bass.ds(rk, 1), :, 128 * k : 128 * (k + 1), :]
                                .rearrange("a i p c -> p (a i) c"),
                            )
                        ch = wchp.tile([128, KT, NC], BF16, tag="wch")
                        for k2 in range(KT // 2):
                            src = ch8[:, 2 * k2 : 2 * k2 + 2, :, :].rearrange(
                                "p a i c -> p a (i c)")
                            dst = ch[:, 2 * k2 : 2 * k2 + 2, :]
                            if k2 % 2 == 0:
                                nc.vector.tensor_copy(dst, src)
                            else:
                                nc.scalar.copy(dst, src)
                        gemm_rank(rank_sync, rank_scalar, ch, pvb_pool, ostage, "rem")
            wchp_ctx.close()
            wp_ctx.close()
            xT_ctx.close()

    nc.finalize()
    return nc


class _Built:
    cache = {}


def _get_built(M, K, N, CAL, n_cores):
    key = (M, K, N, CAL, n_cores)
    if key not in _Built.cache:
        _Built.cache[key] = build_bass(M, K, N, CAL, n_cores)
    return _Built.cache[key]


def make_in_maps(x, weight, bias, calibration, n_cores):
    C = n_cores
    M = x.shape[0]
    N = weight.shape[0]
    CAL = calibration.shape[0]
    MC, NC, CALC = M // C, N // C, CAL // C
    x = np.ascontiguousarray(x, dtype=np.float32)
    weight = np.ascontiguousarray(weight, dtype=np.float32)
    bias = np.ascontiguousarray(bias, dtype=np.float32)
    calibration = np.ascontiguousarray(calibration, dtype=np.float32)
    return [
        {
            "x": x[c * MC : (c + 1) * MC],
            "w": weight[c * NC : (c + 1) * NC],
            "cal": calibration[c * CALC : (c + 1) * CALC],
            "bias": bias,
        }
        for c in range(C)
    ]


def kernel(x, weight, bias, calibration):
    n_cores = 8
    M, K = x.shape
    N = weight.shape[0]
    CAL = calibration.shape[0]
    nc = _get_built(M, K, N, CAL, n_cores)
    in_maps = make_in_maps(x, weight, bias, calibration, n_cores)
    res = run_bass_kernel_spmd(nc, in_maps, list(range(n_cores)))
    out = np.concatenate([res.results[c]["out"] for c in range(n_cores)], axis=0)
    return out.astype(np.float32)
